# revision 13
# baseline (speedup 1.0000x reference)
"""Koopman operator propagation kernel for Trainium2 (Bass/Tile), 8 NeuronCores.

v6: fully step-fused fp8 DoubleRow formulation; the device computes only
the UPDATE, the host adds it to the exact fp32 state.

    z_s = z0 + Delta,
    Delta = (M^s - I) z0 + E U (a . (V^T G z0)),   M = I + DT*A,
    G = mean_k M^k,  E = sum_k M^(s-1-k),  k = 0..s-1.

Cross terms are O(|DT*B|^2); the per-step spread around G cancels to first
order (G is the group mean). Keeping z0 host-side removes the PSUM seed
entirely: z reaches the device as ONE e4m3 plane, the PSUM accumulator
holds S*Delta, and the bf16 Delta output is added to z0 in fp32 on the
host. Numpy sim of the exact scheme: 5.3e-3 max rel err (gate 2e-2).

Per column tile (512 rows x 256 features): 5 fp8 DoubleRow matmuls
(256-deep contraction at 0.5 PE cycles/row), 1 DVE multiply; one ACT copy
per PSUM half per 3-tile group. The delta leaves the device as scaled
e4m3 (sim err incl. this: 9.0e-3, gate 2e-2). DMAs span 6-tile
super-groups: 3 DMAs per 6 tiles (SP queue cost ~0.7us per DMA).
"""

import numpy as np

P = 128
M = 256            # latent dim
DA = 6             # action dim
R = 16             # low-rank dim
J = DA * R         # 96 concatenated rank columns
B_FULL = 4096
T_FULL = 64
NFULL = B_FULL * T_FULL   # 262144 flattened rows
NCORES = 8
NC_ROWS = NFULL // NCORES  # 32768 rows per core
NT = 512           # column-tile width (one PSUM bank of fp32)
NTILES = NC_ROWS // NT     # 64
GRP = 3            # column tiles per DMA group (PSUM: 6 master + 2 pp banks)
DT = 0.1
B_MAX = 0.3

SGRP = 2 * GRP         # column tiles per DMA super-group
S_MASTER = 2.0 ** 10   # PSUM accumulator scale
SV = 2.0 ** 6          # V factor scale
SU = 2.0 ** 8          # U factor scale
SA = S_MASTER / (SV * SU)  # folded into the a expansion
S_OUT = 2.0 ** 3       # e4m3 delta output scale (host divides)

_CACHE = {}
_LAST_RESULT = None


def _build(steps: int):
    from contextlib import ExitStack

    import concourse.mybir as mybir
    import concourse.tile as tile
    from concourse import bacc

    f32 = mybir.dt.float32
    fp8 = mybir.dt.float8e4
    mult = mybir.AluOpType.mult
    DR = mybir.MatmulPerfMode.DoubleRow
    OUT_MUL = S_OUT / S_MASTER

    nc = bacc.Bacc("TRN2", target_bir_lowering=False, num_devices=NCORES)
    # zq[p, c, n] = e4m3(z)[c*128+p, n]
    zq = nc.declare_dram_parameter("zq", [P, 2, NC_ROWS], fp8, isOutput=False)
    aexp = nc.declare_dram_parameter("aexp", [J, NC_ROWS], fp8, isOutput=False)
    # wM[p, c, mo] = S*(M^steps - I)[mo, c*128+p]
    wM = nc.declare_dram_parameter("wM", [P, 2, M], fp8, isOutput=False)
    # wV[p, c, j] = SV*(G.T @ Vcat)[c*128+p, j]
    wV = nc.declare_dram_parameter("wV", [P, 2, J], fp8, isOutput=False)
    # wU[j, pl, mo] = SU*DT*(Ucat @ E.T)[j, mo] / 2   (both planes)
    wU = nc.declare_dram_parameter("wU", [J, 2, M], fp8, isOutput=False)
    dO = nc.declare_dram_parameter("dO", [M, NC_ROWS], fp8, isOutput=True)

    dOr = dO[:].rearrange("(c p) n -> p c n", p=P)
    GW = GRP * NT
    SW = SGRP * NT

    with tile.TileContext(nc) as tc, ExitStack() as ctx:
        wpool = ctx.enter_context(tc.tile_pool(name="w", bufs=1))
        zqpool = ctx.enter_context(tc.tile_pool(name="zq", bufs=2))
        apool = ctx.enter_context(tc.tile_pool(name="a", bufs=2))
        dpool = ctx.enter_context(tc.tile_pool(name="d", bufs=2 * GRP))
        opool = ctx.enter_context(tc.tile_pool(name="o", bufs=2))
        psz = ctx.enter_context(tc.tile_pool(name="psz", bufs=1, space="PSUM"))
        psp = ctx.enter_context(tc.tile_pool(name="psp", bufs=2, space="PSUM"))

        wm = wpool.tile([P, 2, M], fp8)
        nc.sync.dma_start(wm[:], wM[:])
        wv = wpool.tile([P, 2, J], fp8)
        nc.sync.dma_start(wv[:], wV[:])
        wu = wpool.tile([J, 2, M], fp8)
        nc.sync.dma_start(wu[:], wU[:])

        nsuper = (NTILES + SGRP - 1) // SGRP
        for sg in range(nsuper):
            st0 = sg * SGRP
            nt_s = min(SGRP, NTILES - st0)
            sn0 = st0 * NT
            sw = nt_s * NT
            zt = zqpool.tile([P, 2, SW], fp8, tag="zq")
            nc.sync.dma_start(zt[:, :, :sw], zq[:, :, sn0:sn0 + sw])
            at = apool.tile([J, SW], fp8, tag="at")
            nc.sync.dma_start(at[:, :sw], aexp[:, sn0:sn0 + sw])
            zoutm = opool.tile([P, 2, SW], fp8, tag="zout")

            for g0 in range(0, nt_s, GRP):
                nt_g = min(GRP, nt_s - g0)
                gbase = g0 * NT
                # group-wide accumulators: one 3-bank PSUM tile per half,
                # each column tile in its own bank-aligned 512 slice
                pzm = [
                    psz.tile([P, GW], f32, tag=f"pz{c}", name=f"pz{c}")
                    for c in (0, 1)
                ]
                tiles = []
                for t in range(nt_g):
                    tiles.append(
                        {"off": gbase + t * NT,
                         "pz": [pzm[c][:, t * NT:(t + 1) * NT]
                                for c in (0, 1)]}
                    )

                def zsl(tl):
                    return zt[:, :, tl["off"]:tl["off"] + NT]

                # V projection + a-multiply (gates the U chain); pp pool
                # caps outstanding projections at 2, so the third V goes
                # after the first M block while scale_A drains.
                def emit_v(tl):
                    pp = psp.tile([J, NT], f32, tag="pp")
                    nc.tensor.matmul(
                        pp[:], wv[:], zsl(tl),
                        start=True, stop=True, perf_mode=DR,
                    )
                    dt_ = dpool.tile([J, NT], fp8, tag="d")
                    nc.vector.tensor_tensor(
                        dt_[:], pp[:], at[:, tl["off"]:tl["off"] + NT], mult
                    )
                    tl["d"] = dt_

                for tl in tiles[:2]:
                    emit_v(tl)
                # accumulator = S*(M^steps - I) z0
                for c in (0, 1):
                    for tl in tiles:
                        nc.tensor.matmul(
                            tl["pz"][c],
                            wm[:, :, c * P:(c + 1) * P],
                            zsl(tl),
                            start=True, stop=False,
                            perf_mode=DR, skip_group_check=True,
                        )
                    if c == 0:
                        for tl in tiles[2:]:
                            emit_v(tl)
                # accumulator += (E U) d  (broadcast planes, halved wU).
                # Copy each PSUM half out right after its last matmul so
                # the copy overlaps the other half's matmuls and the next
                # group's M-term reclaims the bank sooner (psz bufs=1).
                gw = nt_g * NT
                for c in (0, 1):
                    for tl in tiles:
                        d3 = tl["d"][:].rearrange(
                            "p (one n) -> p one n", one=1
                        ).broadcast_to((J, 2, NT))
                        nc.tensor.matmul(
                            tl["pz"][c],
                            wu[:, :, c * P:(c + 1) * P],
                            d3,
                            start=False, stop=True,
                            perf_mode=DR, skip_group_check=True,
                        )
                    # alternate the second copy onto DVE so the ACT copy
                    # wall overlaps better (both engines stay under PE)
                    if c == 1 and (sg * 2 + g0 // GRP) % 2:
                        nc.vector.tensor_scalar(
                            zoutm[:, c, gbase:gbase + gw], pzm[c][:, :gw],
                            OUT_MUL, None, mult,
                        )
                    else:
                        nc.scalar.mul(
                            zoutm[:, c, gbase:gbase + gw], pzm[c][:, :gw],
                            OUT_MUL,
                        )
            nc.sync.dma_start(dOr[:, :, sn0:sn0 + sw], zoutm[:, :, :sw])
    nc.finalize()
    return nc


def _prep_weights(A, B_U, B_V, steps):
    """DT, tanh clamp, fp8 range scales, and M^k powers folded on host."""
    import ml_dtypes

    e4 = ml_dtypes.float8_e4m3
    A64 = np.asarray(A, np.float64)
    Uc = np.tanh(np.asarray(B_U, np.float64)) * B_MAX   # (6, 256, 16)
    Vc = np.tanh(np.asarray(B_V, np.float64)) * B_MAX
    Vcat = Vc.transpose(1, 0, 2).reshape(M, J)
    Ucat = Uc.transpose(0, 2, 1).reshape(J, M)
    Mm = np.eye(M) + DT * A64
    Mp = [np.linalg.matrix_power(Mm, k) for k in range(steps + 1)]
    G = sum(Mp[k] for k in range(steps)) / steps
    E = sum(Mp[steps - 1 - k] for k in range(steps))

    wM_ = np.ascontiguousarray(
        (S_MASTER * (Mp[steps] - np.eye(M))).T.reshape(2, P, M).transpose(1, 0, 2)
    ).astype(e4)
    wV_ = np.ascontiguousarray(
        (SV * (G.T @ Vcat)).reshape(2, P, J).transpose(1, 0, 2)
    ).astype(e4)
    wU_ = np.empty((J, 2, M), dtype=e4)
    Eh = (SU * DT * (Ucat @ E.T)) / 2.0
    wU_[:, 0, :] = Eh.astype(e4)
    wU_[:, 1, :] = Eh.astype(e4)
    return wM_, wV_, wU_


def kernel(z, a, A, B_U, B_V, steps):
    from concourse.bass_utils import run_bass_kernel_spmd

    steps = int(steps)
    z = np.asarray(z, np.float32)
    out_shape = z.shape
    if steps == 0:
        return z.copy()

    z_f = z.reshape(-1, M)
    a_f = np.asarray(a, np.float32).reshape(-1, DA)
    wM_, wV_, wU_ = _prep_weights(A, B_U, B_V, steps)

    import ml_dtypes
    e4 = ml_dtypes.float8_e4m3
    zT = np.ascontiguousarray(z_f.T)                              # (256, N)
    # zq[p, c, n] = e4m3(z)[c*128+p, n]
    zq = np.ascontiguousarray(zT.astype(e4).reshape(2, P, NFULL).transpose(1, 0, 2))
    aex = np.ascontiguousarray(
        np.repeat(a_f.T * np.float32(SA), R, axis=0).astype(e4)
    )

    if steps not in _CACHE:
        _CACHE[steps] = _build(steps)
    nc = _CACHE[steps]

    in_maps = []
    for c in range(NCORES):
        sl = slice(c * NC_ROWS, (c + 1) * NC_ROWS)
        in_maps.append(
            {
                "zq": np.ascontiguousarray(zq[:, :, sl]),
                "aexp": np.ascontiguousarray(aex[:, sl]),
                "wM": wM_,
                "wV": wV_,
                "wU": wU_,
            }
        )

    res = run_bass_kernel_spmd(nc, in_maps, core_ids=list(range(NCORES)))
    global _LAST_RESULT
    _LAST_RESULT = res
    do = np.concatenate([res.results[c]["dO"] for c in range(NCORES)], axis=1)
    out = z_f + do.T.astype(np.float32) * np.float32(1.0 / S_OUT)
    return np.ascontiguousarray(out).reshape(out_shape)
